# revision 36
# baseline (speedup 1.0000x reference)
"""Causal self-attention block (RMSNorm + QKV + RoPE + causal attention +
out-proj + residual) on 8 Trainium2 NeuronCores.

Sharding: batch (B=2) x head-groups (16 heads -> 4 groups of 4) = 8 shards.
Core c handles batch b = c // 4 and heads [4*(c%4), 4*(c%4)+4).
Each core computes RMSNorm(x_b), its 4 heads' Q/K/V projections, RoPE,
causal attention, and a partial out-projection over its 256-dim slice of
the concatenated head outputs.  The host sums the 4 partials per batch and
adds the residual (the reduction the sharding_hint's "all-reduce after
out_proj" refers to, done during the host-side gather).

Layout notes:
 - All attention operands live transposed (head_dim on partitions):
   Q^T/K^T are built by PE transposes of the projection output; RoPE is
   applied in the transposed domain with per-partition cos/sin tables and
   a second "rotate-half-permuted" PE transpose.
 - scores^T (k on partitions, q free) lets softmax skip max-subtraction
   (scores are O(3) here) and the ones-column appended to V yields the
   softmax denominators from the same PV matmul.
 - Work is emitted in q-chunk groups (A: proj for 4 t-tiles -> B: all
   heads' attention for that q-chunk -> C: out-proj) so the Tile
   scheduler can overlap phases and keep the PE warm.

Self-contained: hardcodes all shapes; no sibling imports.
"""

import numpy as np

import ml_dtypes

import concourse.bacc as bacc
import concourse.tile as tile
from concourse import mybir
from concourse.bass_utils import run_bass_kernel_spmd
from concourse.masks import make_identity

# Problem shapes (hardcoded per contract)
B, T, D, NHEADS = 2, 2048, 1024, 16
HEAD_DIM = 64
EPS = 1e-6
ROPE_BASE = 10000.0

HL = 4          # heads per core
E3 = 3 * HL * HEAD_DIM  # 768 local qkv output dims
P = 128
NT = T // P     # 16 t-tiles
ND = D // P     # 8 d-tiles of the model dim
NQC = T // 512  # 4 query chunks
NCORES = 8

F32 = mybir.dt.float32
F32R = mybir.dt.float32r
BF16 = mybir.dt.bfloat16

# Matmul operand dtype. bf16 streams 1 cycle/row on the PE (f32r takes 2,
# f32 takes 4) and halves SBUF/DMA for the attention operands.
MM_DT = BF16
TWO_BYTE = MM_DT == BF16
# dtype for x / rope tables (host converts)
X_DT = BF16 if TWO_BYTE else F32
# rotate-half permutation via a negative-step AP on the transpose weights;
# set False to use 4 explicit 32-column sub-transposes instead.
PERM_NEG_STEP = False


def _r(ap):
    """View an AP as the matmul streaming dtype."""
    if ap.dtype == MM_DT:
        return ap
    return ap.bitcast(MM_DT)


def _build_program():
    """Emit the per-core Bass/Tile program (identical on all 8 cores)."""
    nc = bacc.Bacc("TRN2", target_bir_lowering=False, debug=False,
                   num_devices=NCORES)

    xb = nc.dram_tensor("xb", [T, D], X_DT, kind="ExternalInput").ap()
    wqkv_t = nc.dram_tensor("wqkv_t", [D, E3], MM_DT, kind="ExternalInput").ap()
    wout_t = nc.dram_tensor("wout_t", [HL * HEAD_DIM, D], MM_DT,
                            kind="ExternalInput").ap()
    cos2 = nc.dram_tensor("cos2", [P, T], X_DT, kind="ExternalInput").ap()
    sin2 = nc.dram_tensor("sin2", [P, T], X_DT, kind="ExternalInput").ap()
    triw = nc.dram_tensor("triw", [P, P], MM_DT, kind="ExternalInput").ap()
    outp = nc.dram_tensor("outp", [T, D], F32, kind="ExternalOutput").ap()

    with tile.TileContext(nc) as tc:
        _emit(tc, xb, wqkv_t, wout_t, cos2, sin2, triw, outp)

    nc.compile()
    return nc


def _emit(tc, xb, wqkv_t, wout_t, cos2, sin2, triw, outp):
    nc = tc.nc
    from contextlib import ExitStack
    ctx = ExitStack()
    with ctx:
        const = ctx.enter_context(tc.tile_pool(name="const", bufs=1))
        persist = ctx.enter_context(tc.tile_pool(name="persist", bufs=1))
        xin = ctx.enter_context(tc.tile_pool(name="xin", bufs=5))
        hrow = ctx.enter_context(tc.tile_pool(name="hrow", bufs=2))
        stats = ctx.enter_context(tc.tile_pool(name="stats", bufs=6))
        htp = ctx.enter_context(tc.tile_pool(name="htp", bufs=10))
        qkrm = ctx.enter_context(tc.tile_pool(name="qkrm", bufs=2))
        rtmp = ctx.enter_context(tc.tile_pool(name="rtmp", bufs=4))
        csin = ctx.enter_context(tc.tile_pool(name="csin", bufs=3))
        ptp = ctx.enter_context(tc.tile_pool(name="ptp", bufs=34))
        nrm = ctx.enter_context(tc.tile_pool(name="nrm", bufs=2))
        orow = ctx.enter_context(tc.tile_pool(name="orow", bufs=3))
        # PSUM budget (8 banks): qkp 2 + vp 1 + pv 1 + sm 2 + tp 2
        psp = ctx.enter_context(
            tc.tile_pool(name="psp", bufs=2, space="PSUM"))

        # ---- constants / weights resident in SBUF ----
        ident = const.tile([P, P], F32)
        make_identity(nc, ident)
        ident_r = const.tile([P, P], MM_DT)
        nc.scalar.copy(ident_r[:], ident[:])
        ident_x = const.tile([P, P], X_DT)
        nc.scalar.copy(ident_x[:], ident[:])
        tri_sb = const.tile([P, P], MM_DT)
        nc.sync.dma_start(out=tri_sb[:], in_=triw[:])
        eps_sb = const.tile([P, 1], F32)
        nc.vector.memset(eps_sb[:], float(EPS))
        zero_sb = const.tile([P, 1], F32)
        nc.vector.memset(zero_sb[:], 0.0)

        wq_sb = persist.tile([P, ND * E3], MM_DT)   # d-block j at cols [E3*j]
        for j in range(ND):
            nc.sync.dma_start(out=wq_sb[:, E3 * j:E3 * (j + 1)],
                              in_=wqkv_t[P * j:P * (j + 1), :])
        wo_sb = persist.tile([P, 2 * D], MM_DT)     # d-block j at cols [D*j]
        for j in range(2):
            nc.sync.dma_start(out=wo_sb[:, D * j:D * (j + 1)],
                              in_=wout_t[P * j:P * (j + 1), :])

        # Q^T per q-chunk: (128, 2*512); blk j at cols [512j], head h at
        # partitions 64*(h%2) of blk h//2, free = t within the chunk.
        qT_c = [persist.tile([P, 2 * 512], MM_DT, name=f"qT{i}", tag=f"qT{i}")
                for i in range(NQC)]
        # K^T per k-tile: (128, 2*128); blk j at cols [128j].
        kT_t = [persist.tile([P, 2 * P], MM_DT, name=f"kT{i}", tag=f"kT{i}")
                for i in range(NT)]
        # V row-major per k-tile with interleaved ones-column per head.
        VW = HL * (HEAD_DIM + 1)  # 260
        v_t = [persist.tile([P, VW], MM_DT, name=f"vT{i}", tag=f"vT{i}")
               for i in range(NT)]
        for ki in range(NT):
            oc = v_t[ki].rearrange("p (h c) -> p h c",
                                   c=HEAD_DIM + 1)[:, :, HEAD_DIM:]
            nc.vector.memset(oc if TWO_BYTE else oc.bitcast(F32), 1.0)
        # attn-out^T per q-chunk (128, 2*512), laid out like qT_c.
        att_c = [persist.tile([P, 2 * 512], MM_DT, name=f"att{i}", tag=f"att{i}")
                 for i in range(NQC)]

        # ---------------- phase bodies ----------------
        def load_stats(ti):
            """DMA x tile and compute its inverse RMS norm (128,1).

            The 1/rms scale factors out of the QKV contraction, so the raw
            x tile feeds the matmul and the scale is applied per-partition
            during the projection evictions."""
            x_t = xin.tile([P, D], X_DT)
            nc.sync.dma_start(out=x_t[:], in_=xb[P * ti:P * (ti + 1), :])
            sq = hrow.tile([P, D], F32, tag="h")
            ssum = stats.tile([P, 1], F32, tag="ssum")
            nc.scalar.activation(sq[:], x_t[:],
                                 mybir.ActivationFunctionType.Square,
                                 accum_out=ssum[:])
            rstd = stats.tile([P, 1], F32, tag="rstd")
            nc.scalar.activation(rstd[:], ssum[:],
                                 mybir.ActivationFunctionType.Sqrt,
                                 bias=eps_sb[:], scale=1.0 / D)
            rinv = stats.tile([P, 1], F32, tag="rinv")
            nc.vector.reciprocal(rinv[:], rstd[:])
            return x_t, rinv

        def phase_a(ti, x_t, rinv):
            """QKV projection + transposed-domain RoPE for one t-tile."""
            # transpose raw x tile -> one (128, 1024) xT psum tile (8 blocks)
            tpx = psp.tile([P, ND * P], X_DT, tag="tp")
            for j in range(ND):
                nc.tensor.transpose(tpx[:, P * j:P * (j + 1)],
                                    x_t[:, P * j:P * (j + 1)], ident_x[:])
            hbig = htp.tile([P, ND * P], MM_DT)
            nc.scalar.copy(hbig[:], tpx[:])

            qk_ps = psp.tile([P, 512], F32, tag="qkp")
            v_ps = psp.tile([P, 256], F32, tag="vp", bufs=1)
            for j in range(ND):
                lhs = _r(hbig[:, P * j:P * (j + 1)])
                nc.tensor.matmul(qk_ps[:], lhs,
                                 _r(wq_sb[:, E3 * j:E3 * j + 512]),
                                 start=(j == 0), stop=(j == ND - 1))
                nc.tensor.matmul(v_ps[:], lhs,
                                 _r(wq_sb[:, E3 * j + 512:E3 * (j + 1)]),
                                 start=(j == 0), stop=(j == ND - 1))

            # evict q,k (ACT, contiguous); rotate-half-shuffled copy and the
            # interleaved V layout go through DVE (strided)
            qk_t = qkrm.tile([P, 512], MM_DT, tag="qkt")
            nc.vector.tensor_scalar_mul(qk_t[:], qk_ps[:], rinv[:])
            qk_s = qkrm.tile([P, 512], MM_DT, tag="qks")
            HH = HEAD_DIM // 2  # 32

            def halves(ap, off):
                return ap.rearrange("p (b i) -> p b i", i=HH)[:, off::2, :]

            nc.vector.tensor_scalar_mul(halves(qk_s, 0), halves(qk_ps[:], 1),
                                        rinv[:])
            nc.vector.tensor_scalar_mul(halves(qk_s, 1), halves(qk_ps[:], 0),
                                        rinv[:])
            vdst = v_t[ti].rearrange("p (h c) -> p h c",
                                     c=HEAD_DIM + 1)[:, :, 0:HEAD_DIM]
            vsrc = v_ps[:].rearrange("p (h c) -> p h c", c=HEAD_DIM)
            nc.vector.tensor_scalar_mul(vdst, vsrc, rinv[:])

            # per-tile cos/sin tables in transposed layout (128, 128)
            ct = csin.tile([P, P], X_DT, tag="ct")
            st = csin.tile([P, P], X_DT, tag="st")
            nc.sync.dma_start(out=ct[:], in_=cos2[:, P * ti:P * (ti + 1)])
            nc.sync.dma_start(out=st[:], in_=sin2[:, P * ti:P * (ti + 1)])

            # transpose q,k blocks into one (128, 1024) psum tile; apply RoPE
            # in the transposed domain:
            #   out[p] = tpA[p]*cos2[p] + tpA[sigma(p)]*sin2'[p]
            # where sigma swaps 32-halves within each head and sin2' carries
            # the rotate-half sign.
            qc, tloc = ti // 4, ti % 4
            tpq = psp.tile([P, ND * P], MM_DT, tag="tp")
            for m in range(4):
                nc.tensor.transpose(tpq[:, 256 * m:256 * m + P],
                                    qk_t[:, P * m:P * (m + 1)], ident_r[:])
                nc.tensor.transpose(tpq[:, 256 * m + P:256 * (m + 1)],
                                    qk_s[:, P * m:P * (m + 1)], ident_r[:])
            for blk in range(2):
                for part, dst in ((0, qT_c[qc]), (1, kT_t[ti])):
                    m = 2 * part + blk
                    tpA = tpq[:, 256 * m:256 * m + P]
                    tpB = tpq[:, 256 * m + P:256 * (m + 1)]
                    if part == 0:
                        dap = dst[:, 512 * blk + P * tloc:512 * blk + P * (tloc + 1)]
                    else:
                        dap = dst[:, P * blk:P * (blk + 1)]
                    tmp = rtmp.tile([P, P], MM_DT, tag="rt")
                    nc.vector.tensor_mul(tmp[:], tpB, st[:])
                    nc.vector.tensor_mul(dap, tpA, ct[:])
                    nc.vector.tensor_add(dap, dap, tmp[:])

        SC = 0.125  # 1/sqrt(64)

        def st_pass(h, qc):
            """Scores + exp for head h / query chunk qc -> list of pt tiles."""
            bp = 64 * (h % 2)
            blk = h // 2
            nki = 4 * qc + 4
            pts = []
            for ki in range(nki):
                zm = max(0, P * ki - 512 * qc)   # first valid column
                # (f32r only) widen to >=256-wide streams; garbage columns
                # [z:zm) get zeroed in pt before the PV matmul.
                z = zm if TWO_BYTE else min(zm, 256)
                st_ps = psp.tile([P, 512], F32, tag="sm")
                nc.tensor.matmul(
                    st_ps[:, z:512],
                    _r(kT_t[ki][bp:bp + 64, P * blk:P * (blk + 1)]),
                    _r(qT_c[qc][bp:bp + 64, 512 * blk + z:512 * (blk + 1)]),
                    start=True, stop=True)
                pt = ptp.tile([P, 512], MM_DT)
                nc.scalar.activation(pt[:, z:512], st_ps[:, z:512],
                                     mybir.ActivationFunctionType.Exp,
                                     bias=zero_sb[:], scale=SC)
                if zm > z:
                    ap0 = pt[:, z:zm] if TWO_BYTE else pt[:, z:zm].bitcast(F32)
                    nc.vector.memset(ap0, 0.0)
                if ki >= 4 * qc:  # diagonal block: apply causal mask
                    nc.gpsimd.tensor_mul(pt[:, zm:zm + P], pt[:, zm:zm + P],
                                         tri_sb[:])
                pts.append((pt, z))
            return pts

        def pv_pass(pts, g):
            """PV accumulation + softmax normalization for group g=(qc,h)."""
            qc, h = g
            bp = 64 * (h % 2)
            blk = h // 2
            nki = len(pts)
            pv_ps = psp.tile([65, 512], F32, tag="pv", bufs=1)
            for ki in range(nki):
                pt, z = pts[ki]
                nc.tensor.matmul(
                    pv_ps[:, z:512],
                    _r(v_t[ki][:, 65 * h:65 * (h + 1)]),
                    _r(pt[:, z:512]),
                    start=(ki == 0), stop=(ki == nki - 1))
            # normalize: rows 0:64 are sum(p*v), row 64 is sum(p)
            srow = nrm.tile([1, 512], F32, tag="srow")
            nc.vector.tensor_copy(srow[:], pv_ps[64:65, :])
            rrow = nrm.tile([1, 512], F32, tag="rrow")
            nc.vector.reciprocal_approx_fast(rrow[:], srow[:])
            bcast = nrm.tile([64, 512], F32, tag="bcast")
            nc.gpsimd.partition_broadcast(bcast[:], rrow[:])
            nc.vector.tensor_mul(
                att_c[qc][bp:bp + 64, 512 * blk:512 * (blk + 1)],
                pv_ps[0:64, :], bcast[:])

        def phase_c(ti):
            """Partial out-projection for one t-tile."""
            qc, tloc = ti // 4, ti % 4
            o_t = orow.tile([P, D], F32)
            for ec in range(2):
                op_ps = psp.tile([P, 512], F32, tag="sm")
                for j in range(2):
                    lhs = att_c[qc][:, 512 * j + P * tloc:512 * j + P * (tloc + 1)]
                    nc.tensor.matmul(
                        op_ps[:], _r(lhs),
                        _r(wo_sb[:, D * j + 512 * ec:D * j + 512 * (ec + 1)]),
                        start=(j == 0), stop=(j == 1))
                nc.vector.tensor_copy(o_t[:, 512 * ec:512 * (ec + 1)], op_ps[:])
            nc.sync.dma_start(out=outp[P * ti:P * (ti + 1), :], in_=o_t[:])

        # ---------------- emission: interleave A/B/C by q-chunk ----------
        # software-pipelined emission: the PV pass of group g-1 is emitted
        # after the ST pass of group g, so the PE has dense PV work while
        # the ACT engine chews through group g's exps.  RMS stats run one
        # tile ahead of the projection work.
        groups = [(qc, h) for qc in range(NQC) for h in range(HL)]
        prev = None
        cur = load_stats(0)
        for g in groups:
            qc, h = g
            if h == 0:
                for ti in range(4 * qc, 4 * qc + 4):
                    nxt = load_stats(ti + 1) if ti + 1 < NT else None
                    phase_a(ti, *cur)
                    cur = nxt
            pts = st_pass(h, qc)
            if prev is not None:
                pv_pass(*prev)
                if prev[1][1] == HL - 1:
                    for ti in range(4 * prev[1][0], 4 * prev[1][0] + 4):
                        phase_c(ti)
            prev = (pts, g)
        pv_pass(*prev)
        for ti in range(4 * (NQC - 1), NT):
            phase_c(ti)


# ---------------- host-side driver ----------------

_CACHE = {}


def _get_program():
    if "nc" not in _CACHE:
        _CACHE["nc"] = _build_program()
    return _CACHE["nc"]


def _rope_tables():
    half = HEAD_DIM // 2
    inv_freq = (1.0 / (ROPE_BASE ** (np.arange(half, dtype=np.float32) / half))
                ).astype(np.float32)
    pos = np.arange(T, dtype=np.float32)
    freqs = pos[:, None] * inv_freq[None, :]
    emb = np.concatenate([freqs, freqs], axis=-1).astype(np.float32)
    return np.cos(emb).astype(np.float32), np.sin(emb).astype(np.float32)


def make_in_maps(x, norm_w, w_qkv, w_out):
    np_mm = ml_dtypes.bfloat16 if TWO_BYTE else np.float32
    np_x = ml_dtypes.bfloat16 if TWO_BYTE else np.float32
    cos, sin = _rope_tables()   # (T, 64) each
    # transposed-domain tables, stacked for two heads per partition block:
    # row p covers head-dim p%64; sin2 carries the rotate-half sign.
    dhidx = np.arange(P) % HEAD_DIM
    sgn = np.where(dhidx < HEAD_DIM // 2, -1.0, 1.0).astype(np.float32)
    cos2 = np.ascontiguousarray(cos.T[dhidx]).astype(np_x)   # (128, T)
    sin2 = np.ascontiguousarray(sin.T[dhidx] * sgn[:, None]).astype(np_x)
    tri = (np.arange(P)[None, :] >= np.arange(P)[:, None]).astype(np_mm)
    w_fold = (w_qkv * norm_w[None, :]).astype(np.float32)
    in_maps = []
    for c in range(NCORES):
        b, hg = c // 4, c % 4
        sl = slice(256 * hg, 256 * (hg + 1))
        wq = w_fold[0 * D:1 * D][sl]
        wk = w_fold[1 * D:2 * D][sl]
        wv = w_fold[2 * D:3 * D][sl]
        wqkv_c = np.ascontiguousarray(
            np.concatenate([wq, wk, wv], axis=0).T).astype(np_mm)
        wout_c = np.ascontiguousarray(w_out[:, sl].T).astype(np_mm)
        in_maps.append({
            "xb": np.ascontiguousarray(x[b]).astype(np_x),
            "wqkv_t": wqkv_c,
            "wout_t": wout_c,
            "cos2": cos2, "sin2": sin2, "triw": tri,
        })
    return in_maps


def assemble(x, results):
    out = np.empty((B, T, D), dtype=np.float32)
    for b in range(B):
        acc = x[b].astype(np.float32).copy()
        for hg in range(4):
            acc += results[4 * b + hg]["outp"]
        out[b] = acc
    return out


def kernel(x, norm_w, w_qkv, w_out, trace=False):
    x = np.asarray(x, dtype=np.float32)
    norm_w = np.asarray(norm_w, dtype=np.float32)
    w_qkv = np.asarray(w_qkv, dtype=np.float32)
    w_out = np.asarray(w_out, dtype=np.float32)
    nc = _get_program()
    in_maps = make_in_maps(x, norm_w, w_qkv, w_out)
    res = run_bass_kernel_spmd(nc, in_maps, core_ids=list(range(NCORES)),
                               trace=trace)
    _CACHE["last_results"] = res
    return assemble(x, res.results)


# revision 37
# speedup vs baseline: 1.5818x; 1.5818x over previous
"""Causal self-attention block (RMSNorm + QKV + RoPE + causal attention +
out-proj + residual) on 8 Trainium2 NeuronCores.

Sharding: batch (B=2) x head-groups (16 heads -> 4 groups of 4) = 8 shards.
Core c handles batch b = c // 4 and heads [4*(c%4), 4*(c%4)+4).
Each core computes RMSNorm(x_b), its 4 heads' Q/K/V projections, RoPE,
causal attention, and a partial out-projection over its 256-dim slice of
the concatenated head outputs.  The host sums the 4 partials per batch and
adds the residual (the reduction the sharding_hint's "all-reduce after
out_proj" refers to, done during the host-side gather).

Layout notes:
 - All attention operands live transposed (head_dim on partitions):
   Q^T/K^T are built by PE transposes of the projection output; RoPE is
   applied in the transposed domain with per-partition cos/sin tables and
   a second "rotate-half-permuted" PE transpose.
 - scores^T (k on partitions, q free) lets softmax skip max-subtraction
   (scores are O(3) here) and the ones-column appended to V yields the
   softmax denominators from the same PV matmul.
 - Work is emitted in q-chunk groups (A: proj for 4 t-tiles -> B: all
   heads' attention for that q-chunk -> C: out-proj) so the Tile
   scheduler can overlap phases and keep the PE warm.

Self-contained: hardcodes all shapes; no sibling imports.
"""

import numpy as np

import ml_dtypes

import concourse.bacc as bacc
import concourse.tile as tile
from concourse import mybir
from concourse.bass_utils import run_bass_kernel_spmd
from concourse.masks import make_identity

# Problem shapes (hardcoded per contract)
B, T, D, NHEADS = 2, 2048, 1024, 16
HEAD_DIM = 64
EPS = 1e-6
ROPE_BASE = 10000.0

HL = 4          # heads per core
E3 = 3 * HL * HEAD_DIM  # 768 local qkv output dims
P = 128
NT = T // P     # 16 t-tiles
ND = D // P     # 8 d-tiles of the model dim
NQC = T // 512  # 4 query chunks
NCORES = 8

F32 = mybir.dt.float32
F32R = mybir.dt.float32r
BF16 = mybir.dt.bfloat16

# Matmul operand dtype. bf16 streams 1 cycle/row on the PE (f32r takes 2,
# f32 takes 4) and halves SBUF/DMA for the attention operands.
MM_DT = BF16
TWO_BYTE = MM_DT == BF16
# dtype for x / rope tables (host converts)
X_DT = BF16 if TWO_BYTE else F32
# rotate-half permutation via a negative-step AP on the transpose weights;
# set False to use 4 explicit 32-column sub-transposes instead.
PERM_NEG_STEP = False


def _r(ap):
    """View an AP as the matmul streaming dtype."""
    if ap.dtype == MM_DT:
        return ap
    return ap.bitcast(MM_DT)


def _build_program():
    """Emit the per-core Bass/Tile program (identical on all 8 cores)."""
    nc = bacc.Bacc("TRN2", target_bir_lowering=False, debug=False,
                   num_devices=NCORES)

    xb = nc.dram_tensor("xb", [T, D], X_DT, kind="ExternalInput").ap()
    wqkv_t = nc.dram_tensor("wqkv_t", [D, E3], MM_DT, kind="ExternalInput").ap()
    wout_t = nc.dram_tensor("wout_t", [HL * HEAD_DIM, D], MM_DT,
                            kind="ExternalInput").ap()
    cos2 = nc.dram_tensor("cos2", [P, T], X_DT, kind="ExternalInput").ap()
    sin2 = nc.dram_tensor("sin2", [P, T], X_DT, kind="ExternalInput").ap()
    triw = nc.dram_tensor("triw", [P, P], MM_DT, kind="ExternalInput").ap()
    outp = nc.dram_tensor("outp", [T, D], F32, kind="ExternalOutput").ap()

    with tile.TileContext(nc) as tc:
        _emit(tc, xb, wqkv_t, wout_t, cos2, sin2, triw, outp)

    nc.compile()
    return nc


def _emit(tc, xb, wqkv_t, wout_t, cos2, sin2, triw, outp):
    nc = tc.nc
    from contextlib import ExitStack
    ctx = ExitStack()
    with ctx:
        const = ctx.enter_context(tc.tile_pool(name="const", bufs=1))
        persist = ctx.enter_context(tc.tile_pool(name="persist", bufs=1))
        xin = ctx.enter_context(tc.tile_pool(name="xin", bufs=5))
        hrow = ctx.enter_context(tc.tile_pool(name="hrow", bufs=2))
        stats = ctx.enter_context(tc.tile_pool(name="stats", bufs=6))
        htp = ctx.enter_context(tc.tile_pool(name="htp", bufs=10))
        qkrm = ctx.enter_context(tc.tile_pool(name="qkrm", bufs=2))
        rtmp = ctx.enter_context(tc.tile_pool(name="rtmp", bufs=4))
        csin = ctx.enter_context(tc.tile_pool(name="csin", bufs=3))
        ptp = ctx.enter_context(tc.tile_pool(name="ptp", bufs=34))
        nrm = ctx.enter_context(tc.tile_pool(name="nrm", bufs=2))
        orow = ctx.enter_context(tc.tile_pool(name="orow", bufs=3))
        # PSUM budget (8 banks): qkp 2 + vp 1 + pv 1 + sm 2 + tp 2
        psp = ctx.enter_context(
            tc.tile_pool(name="psp", bufs=2, space="PSUM"))

        # ---- constants / weights resident in SBUF ----
        ident = const.tile([P, P], F32)
        make_identity(nc, ident)
        ident_r = const.tile([P, P], MM_DT)
        nc.scalar.copy(ident_r[:], ident[:])
        ident_x = const.tile([P, P], X_DT)
        nc.scalar.copy(ident_x[:], ident[:])
        tri_sb = const.tile([P, P], MM_DT)
        nc.sync.dma_start(out=tri_sb[:], in_=triw[:])
        eps_sb = const.tile([P, 1], F32)
        nc.vector.memset(eps_sb[:], float(EPS))
        zero_sb = const.tile([P, 1], F32)
        nc.vector.memset(zero_sb[:], 0.0)

        wq_sb = persist.tile([P, ND * E3], MM_DT)   # d-block j at cols [E3*j]
        for j in range(ND):
            nc.sync.dma_start(out=wq_sb[:, E3 * j:E3 * (j + 1)],
                              in_=wqkv_t[P * j:P * (j + 1), :])
        wo_sb = persist.tile([P, 2 * D], MM_DT)     # d-block j at cols [D*j]
        for j in range(2):
            nc.sync.dma_start(out=wo_sb[:, D * j:D * (j + 1)],
                              in_=wout_t[P * j:P * (j + 1), :])

        # Q^T per q-chunk: (128, 2*512); blk j at cols [512j], head h at
        # partitions 64*(h%2) of blk h//2, free = t within the chunk.
        qT_c = [persist.tile([P, 2 * 512], MM_DT, name=f"qT{i}", tag=f"qT{i}")
                for i in range(NQC)]
        # K^T per k-tile: (128, 2*128); blk j at cols [128j].
        kT_t = [persist.tile([P, 2 * P], MM_DT, name=f"kT{i}", tag=f"kT{i}")
                for i in range(NT)]
        # V row-major per k-tile with interleaved ones-column per head.
        VW = HL * (HEAD_DIM + 1)  # 260
        v_t = [persist.tile([P, VW], MM_DT, name=f"vT{i}", tag=f"vT{i}")
               for i in range(NT)]
        for ki in range(NT):
            oc = v_t[ki].rearrange("p (h c) -> p h c",
                                   c=HEAD_DIM + 1)[:, :, HEAD_DIM:]
            nc.vector.memset(oc if TWO_BYTE else oc.bitcast(F32), 1.0)
        # attn-out^T per q-chunk (128, 2*512), laid out like qT_c.
        att_c = [persist.tile([P, 2 * 512], MM_DT, name=f"att{i}", tag=f"att{i}")
                 for i in range(NQC)]

        # ---------------- phase bodies ----------------
        def load_stats(ti):
            """DMA x tile and compute its inverse RMS norm (128,1).

            The 1/rms scale factors out of the QKV contraction, so the raw
            x tile feeds the matmul and the scale is applied per-partition
            during the projection evictions."""
            x_t = xin.tile([P, D], X_DT)
            nc.sync.dma_start(out=x_t[:], in_=xb[P * ti:P * (ti + 1), :])
            sq = hrow.tile([P, D], F32, tag="h")
            ssum = stats.tile([P, 1], F32, tag="ssum")
            nc.scalar.activation(sq[:], x_t[:],
                                 mybir.ActivationFunctionType.Square,
                                 accum_out=ssum[:])
            rstd = stats.tile([P, 1], F32, tag="rstd")
            nc.scalar.activation(rstd[:], ssum[:],
                                 mybir.ActivationFunctionType.Sqrt,
                                 bias=eps_sb[:], scale=1.0 / D)
            rinv = stats.tile([P, 1], F32, tag="rinv")
            nc.vector.reciprocal(rinv[:], rstd[:])
            return x_t, rinv

        def phase_a(ti, x_t, rinv):
            """QKV projection + transposed-domain RoPE for one t-tile."""
            # transpose raw x tile -> one (128, 1024) xT psum tile (8 blocks)
            tpx = psp.tile([P, ND * P], X_DT, tag="tp")
            for j in range(ND):
                nc.tensor.transpose(tpx[:, P * j:P * (j + 1)],
                                    x_t[:, P * j:P * (j + 1)], ident_x[:])
            hbig = htp.tile([P, ND * P], MM_DT)
            nc.scalar.copy(hbig[:], tpx[:])

            qk_ps = psp.tile([P, 512], F32, tag="qkp")
            v_ps = psp.tile([P, 256], F32, tag="vp", bufs=1)
            for j in range(ND):
                lhs = _r(hbig[:, P * j:P * (j + 1)])
                nc.tensor.matmul(qk_ps[:], lhs,
                                 _r(wq_sb[:, E3 * j:E3 * j + 512]),
                                 start=(j == 0), stop=(j == ND - 1))
                nc.tensor.matmul(v_ps[:], lhs,
                                 _r(wq_sb[:, E3 * j + 512:E3 * (j + 1)]),
                                 start=(j == 0), stop=(j == ND - 1))

            # evict q,k (ACT, contiguous); rotate-half-shuffled copy and the
            # interleaved V layout go through DVE (strided)
            qk_t = qkrm.tile([P, 512], MM_DT, tag="qkt")
            nc.vector.tensor_scalar_mul(qk_t[:], qk_ps[:], rinv[:])
            qk_s = qkrm.tile([P, 512], MM_DT, tag="qks")
            HH = HEAD_DIM // 2  # 32

            def halves(ap, off):
                return ap.rearrange("p (b i) -> p b i", i=HH)[:, off::2, :]

            nc.vector.tensor_scalar_mul(halves(qk_s, 0), halves(qk_ps[:], 1),
                                        rinv[:])
            nc.vector.tensor_scalar_mul(halves(qk_s, 1), halves(qk_ps[:], 0),
                                        rinv[:])
            vdst = v_t[ti].rearrange("p (h c) -> p h c",
                                     c=HEAD_DIM + 1)[:, :, 0:HEAD_DIM]
            vsrc = v_ps[:].rearrange("p (h c) -> p h c", c=HEAD_DIM)
            nc.vector.tensor_scalar_mul(vdst, vsrc, rinv[:])

            # per-tile cos/sin tables in transposed layout (128, 128)
            ct = csin.tile([P, P], X_DT, tag="ct")
            st = csin.tile([P, P], X_DT, tag="st")
            nc.sync.dma_start(out=ct[:], in_=cos2[:, P * ti:P * (ti + 1)])
            nc.sync.dma_start(out=st[:], in_=sin2[:, P * ti:P * (ti + 1)])

            # transpose q,k blocks into one (128, 1024) psum tile; apply RoPE
            # in the transposed domain:
            #   out[p] = tpA[p]*cos2[p] + tpA[sigma(p)]*sin2'[p]
            # where sigma swaps 32-halves within each head and sin2' carries
            # the rotate-half sign.
            qc, tloc = ti // 4, ti % 4
            tpq = psp.tile([P, ND * P], MM_DT, tag="tp")
            for m in range(4):
                nc.tensor.transpose(tpq[:, 256 * m:256 * m + P],
                                    qk_t[:, P * m:P * (m + 1)], ident_r[:])
                nc.tensor.transpose(tpq[:, 256 * m + P:256 * (m + 1)],
                                    qk_s[:, P * m:P * (m + 1)], ident_r[:])
            for blk in range(2):
                for part, dst in ((0, qT_c[qc]), (1, kT_t[ti])):
                    m = 2 * part + blk
                    tpA = tpq[:, 256 * m:256 * m + P]
                    tpB = tpq[:, 256 * m + P:256 * (m + 1)]
                    if part == 0:
                        dap = dst[:, 512 * blk + P * tloc:512 * blk + P * (tloc + 1)]
                    else:
                        dap = dst[:, P * blk:P * (blk + 1)]
                    tmp = rtmp.tile([P, P], MM_DT, tag="rt")
                    nc.vector.tensor_mul(tmp[:], tpB, st[:])
                    nc.vector.tensor_mul(dap, tpA, ct[:])
                    nc.vector.tensor_add(dap, dap, tmp[:])

        SC = 0.125  # 1/sqrt(64)

        def st_pass(h, qc):
            """Scores + exp for head h / query chunk qc -> list of pt tiles."""
            bp = 64 * (h % 2)
            blk = h // 2
            nki = 4 * qc + 4
            pts = []
            for ki in range(nki):
                zm = max(0, P * ki - 512 * qc)   # first valid column
                # (f32r only) widen to >=256-wide streams; garbage columns
                # [z:zm) get zeroed in pt before the PV matmul.
                z = zm if TWO_BYTE else min(zm, 256)
                st_ps = psp.tile([P, 512], F32, tag="sm")
                nc.tensor.matmul(
                    st_ps[:, z:512],
                    _r(kT_t[ki][bp:bp + 64, P * blk:P * (blk + 1)]),
                    _r(qT_c[qc][bp:bp + 64, 512 * blk + z:512 * (blk + 1)]),
                    start=True, stop=True)
                pt = ptp.tile([P, 512], MM_DT)
                nc.scalar.activation(pt[:, z:512], st_ps[:, z:512],
                                     mybir.ActivationFunctionType.Exp,
                                     bias=zero_sb[:], scale=SC)
                if zm > z:
                    ap0 = pt[:, z:zm] if TWO_BYTE else pt[:, z:zm].bitcast(F32)
                    nc.vector.memset(ap0, 0.0)
                if ki >= 4 * qc:  # diagonal block: apply causal mask
                    nc.vector.tensor_mul(pt[:, zm:zm + P], pt[:, zm:zm + P],
                                         tri_sb[:])
                pts.append((pt, z))
            return pts

        def pv_pass(pts, g):
            """PV accumulation + softmax normalization for group g=(qc,h)."""
            qc, h = g
            bp = 64 * (h % 2)
            blk = h // 2
            nki = len(pts)
            pv_ps = psp.tile([65, 512], F32, tag="pv", bufs=1)
            for ki in range(nki):
                pt, z = pts[ki]
                nc.tensor.matmul(
                    pv_ps[:, z:512],
                    _r(v_t[ki][:, 65 * h:65 * (h + 1)]),
                    _r(pt[:, z:512]),
                    start=(ki == 0), stop=(ki == nki - 1))
            # normalize: rows 0:64 are sum(p*v), row 64 is sum(p)
            srow = nrm.tile([1, 512], F32, tag="srow")
            nc.vector.tensor_copy(srow[:], pv_ps[64:65, :])
            rrow = nrm.tile([1, 512], F32, tag="rrow")
            nc.vector.reciprocal_approx_fast(rrow[:], srow[:])
            bcast = nrm.tile([64, 512], F32, tag="bcast")
            nc.gpsimd.partition_broadcast(bcast[:], rrow[:])
            nc.vector.tensor_mul(
                att_c[qc][bp:bp + 64, 512 * blk:512 * (blk + 1)],
                pv_ps[0:64, :], bcast[:])

        def phase_c(ti):
            """Partial out-projection for one t-tile."""
            qc, tloc = ti // 4, ti % 4
            o_t = orow.tile([P, D], F32)
            for ec in range(2):
                op_ps = psp.tile([P, 512], F32, tag="qkp")
                for j in range(2):
                    lhs = att_c[qc][:, 512 * j + P * tloc:512 * j + P * (tloc + 1)]
                    nc.tensor.matmul(
                        op_ps[:], _r(lhs),
                        _r(wo_sb[:, D * j + 512 * ec:D * j + 512 * (ec + 1)]),
                        start=(j == 0), stop=(j == 1))
                nc.vector.tensor_copy(o_t[:, 512 * ec:512 * (ec + 1)], op_ps[:])
            nc.sync.dma_start(out=outp[P * ti:P * (ti + 1), :], in_=o_t[:])

        # ---------------- emission: interleave A/B/C by q-chunk ----------
        # software-pipelined emission: the PV pass of group g-1 is emitted
        # after the ST pass of group g, so the PE has dense PV work while
        # the ACT engine chews through group g's exps.  RMS stats run one
        # tile ahead of the projection work.
        groups = [(qc, h) for qc in range(NQC) for h in range(HL)]
        prev = None
        cur = load_stats(0)
        for g in groups:
            qc, h = g
            if h == 0:
                for ti in range(4 * qc, 4 * qc + 4):
                    nxt = load_stats(ti + 1) if ti + 1 < NT else None
                    phase_a(ti, *cur)
                    cur = nxt
            pts = st_pass(h, qc)
            if prev is not None:
                pv_pass(*prev)
                if prev[1][1] == HL - 1:
                    for ti in range(4 * prev[1][0], 4 * prev[1][0] + 4):
                        phase_c(ti)
            prev = (pts, g)
        pv_pass(*prev)
        for ti in range(4 * (NQC - 1), NT):
            phase_c(ti)


# ---------------- host-side driver ----------------

_CACHE = {}


def _get_program():
    if "nc" not in _CACHE:
        _CACHE["nc"] = _build_program()
    return _CACHE["nc"]


def _rope_tables():
    half = HEAD_DIM // 2
    inv_freq = (1.0 / (ROPE_BASE ** (np.arange(half, dtype=np.float32) / half))
                ).astype(np.float32)
    pos = np.arange(T, dtype=np.float32)
    freqs = pos[:, None] * inv_freq[None, :]
    emb = np.concatenate([freqs, freqs], axis=-1).astype(np.float32)
    return np.cos(emb).astype(np.float32), np.sin(emb).astype(np.float32)


def make_in_maps(x, norm_w, w_qkv, w_out):
    np_mm = ml_dtypes.bfloat16 if TWO_BYTE else np.float32
    np_x = ml_dtypes.bfloat16 if TWO_BYTE else np.float32
    cos, sin = _rope_tables()   # (T, 64) each
    # transposed-domain tables, stacked for two heads per partition block:
    # row p covers head-dim p%64; sin2 carries the rotate-half sign.
    dhidx = np.arange(P) % HEAD_DIM
    sgn = np.where(dhidx < HEAD_DIM // 2, -1.0, 1.0).astype(np.float32)
    cos2 = np.ascontiguousarray(cos.T[dhidx]).astype(np_x)   # (128, T)
    sin2 = np.ascontiguousarray(sin.T[dhidx] * sgn[:, None]).astype(np_x)
    tri = (np.arange(P)[None, :] >= np.arange(P)[:, None]).astype(np_mm)
    w_fold = (w_qkv * norm_w[None, :]).astype(np.float32)
    in_maps = []
    for c in range(NCORES):
        b, hg = c // 4, c % 4
        sl = slice(256 * hg, 256 * (hg + 1))
        wq = w_fold[0 * D:1 * D][sl]
        wk = w_fold[1 * D:2 * D][sl]
        wv = w_fold[2 * D:3 * D][sl]
        wqkv_c = np.ascontiguousarray(
            np.concatenate([wq, wk, wv], axis=0).T).astype(np_mm)
        wout_c = np.ascontiguousarray(w_out[:, sl].T).astype(np_mm)
        in_maps.append({
            "xb": np.ascontiguousarray(x[b]).astype(np_x),
            "wqkv_t": wqkv_c,
            "wout_t": wout_c,
            "cos2": cos2, "sin2": sin2, "triw": tri,
        })
    return in_maps


def assemble(x, results):
    out = np.empty((B, T, D), dtype=np.float32)
    for b in range(B):
        acc = x[b].astype(np.float32).copy()
        for hg in range(4):
            acc += results[4 * b + hg]["outp"]
        out[b] = acc
    return out


def kernel(x, norm_w, w_qkv, w_out, trace=False):
    x = np.asarray(x, dtype=np.float32)
    norm_w = np.asarray(norm_w, dtype=np.float32)
    w_qkv = np.asarray(w_qkv, dtype=np.float32)
    w_out = np.asarray(w_out, dtype=np.float32)
    nc = _get_program()
    in_maps = make_in_maps(x, norm_w, w_qkv, w_out)
    res = run_bass_kernel_spmd(nc, in_maps, core_ids=list(range(NCORES)),
                               trace=trace)
    _CACHE["last_results"] = res
    return assemble(x, res.results)
